# revision 22
# baseline (speedup 1.0000x reference)
"""Trainium2 Bass kernel for nn_DistanceModule.

Computes, for h [4,512,64], W [64,64], b/gamma/beta [64]:
    x = LayerNorm(ReLU(h @ W.T + b))          # [B,N,C]
    D[b,i,j,c] = x[b,i,c] * x[b,j,c]
    out = softmax(D, axis=-1)                 # [B,N,N,C] f32 (256 MB)

Sharding + symmetry: out[b,i,j,c] == out[b,j,i,c] exactly (the product
commutes), so of each batch's 4x4 grid of 128x128 (i,j) blocks only 12
need computing. Core pair (2b, 2b+1): even core takes tile order
(0,1,2,3), odd core (3,2,1,0) -- the SAME program in local tile
coords computes units [(it0,j 0:256), (it0,j 256:512), (it1,j 128:384)]
on both, which lands on blocks rows 0-1 (even) / rows 3-2 (odd).
The host places 12 blocks directly and 4 as transposes.

Per-core pipeline, c-major on-chip layout (contiguous APs everywhere):
  PE     : K=1 outer-product bf16 matmuls (lhsT = xTcat[0, c*N+i0]
           [1,128], rhs = xTcat[0, c*N+j0] [1,512|256]) write logit
           x_i[c]*x_j[c] into PSUM. All x rows are concatenated on
           partition 0 (matmul base-partition rule) via a DRAM bounce.
  ScalarE: contiguous exp activations (FD=1024, PSUM src, bf16 dst).
           Only Ln/Exp are used on ScalarE (single table set).
  VectorE: channel sums via bf16 add-trees in 16-channel partial
           groups (tensor_tensor 2x 16-bit mode), emitted as the exp
           tiles fill so VectorE overlaps the fill; then
           reciprocal_approx_fast and an in-place bf16 normalize
           multiply against a stride-0-broadcast reciprocal.
  DMA    : contiguous bf16 stores in (c,j) order; the host transposes
           blocks to (j,c) while casting back to f32.

Softmax needs no max-subtraction: LayerNorm bounds |x| by sqrt(C-1),
logits <= 63, exp <= 2.4e27 which fits bf16 range.
Measured rel err ~5e-3 vs f32 reference (harness gate 2e-2).
"""

import numpy as np

import concourse.bacc as bacc
import concourse.bass as bass
import concourse.mybir as mybir
import concourse.tile as tile
from concourse.bass_utils import run_bass_kernel_spmd

B, N, C = 4, 512, 64
NCORES = 8
ROWS = 256
JW = 256            # unit j-width
NU = 3              # units per core
EPS = 1e-5
F32 = mybir.dt.float32
BF16 = mybir.dt.bfloat16

_CACHE = {}


def _build_program():
    nc = bacc.Bacc(
        "TRN2",
        target_bir_lowering=False,
        debug=False,
        enable_asserts=False,
        num_devices=NCORES,
    )

    hT_d = nc.dram_tensor("hT", [C, N], F32, kind="ExternalInput")
    WT_d = nc.dram_tensor("WT", [C, C], F32, kind="ExternalInput")
    bgb_d = nc.dram_tensor("bgb", [128, 3 * C], F32, kind="ExternalInput")
    id_d = nc.dram_tensor("identity", [128, 128], F32, kind="ExternalInput")
    xstage_d = nc.dram_tensor("xstage", [C, N], BF16, kind="Internal")
    # unit u -> rows [u*128,(u+1)*128), (c,j)-major columns
    out_d = nc.dram_tensor("out", [NU * 128, C * JW], BF16, kind="ExternalOutput")

    sub = mybir.AluOpType.subtract
    mult = mybir.AluOpType.mult
    Exp = mybir.ActivationFunctionType.Exp
    Ln = mybir.ActivationFunctionType.Ln

    with tile.TileContext(nc) as tc:
        with tc.tile_pool(name="const", bufs=1) as constp:
            hT = constp.tile([C, N], F32)
            nc.sync.dma_start(hT[:], hT_d[:])
            WT = constp.tile([C, C], F32)
            nc.sync.dma_start(WT[:], WT_d[:])
            bgb = constp.tile([128, 3 * C], F32)
            nc.sync.dma_start(bgb[:], bgb_d[:])
            ident = constp.tile([128, 128], F32)
            nc.sync.dma_start(ident[:], id_d[:])

            xT = constp.tile([C, N], BF16)
            eps_t = constp.tile([128, 1], F32)
            nc.vector.memset(eps_t[:], EPS)
            dummy = constp.tile([1, N], BF16)
            nc.vector.memset(dummy[:], 1.0)

            # ---- x = LayerNorm(ReLU(h @ W.T + b)), transposed to bf16 ----
            # ReLU on VectorE; rstd = exp(-0.5*ln(var+eps)) keeps ScalarE in
            # the natural_log_exp table set (one ACT_TABLE_LOAD).
            with (
                tc.tile_pool(name="xprep", bufs=2) as xprep,
                tc.tile_pool(name="psum_prep", bufs=2, space=bass.MemorySpace.PSUM) as psp,
            ):
                # PE HAM warmup: ~10us of back-to-back K=1 bf16 dummy
                # matmuls (the same shape as the product matmuls) while the
                # input DMAs land and the LN prep runs, so the clock gate
                # reaches 8/8 before the product matmuls start and does not
                # re-cool during the xTcat bounce (PE bursts in the main
                # loop are shorter than the 4us the HAM needs to warm).
                warm = psp.tile([128, N], F32, tag="warm")
                for _ in range(12):
                    nc.tensor.matmul(warm[:], dummy[0:1, 0:128], dummy[0:1, :])
                for t in range(4):
                    xp = psp.tile([128, C], F32, tag="xp")
                    nc.tensor.matmul(xp[:], hT[:, t * 128:(t + 1) * 128], WT[:])
                    xs = xprep.tile([128, C], F32, tag="xs")
                    nc.vector.tensor_add(xs[:], xp[:], bgb[:, 0:C])      # + b
                    nc.vector.tensor_scalar_max(xs[:], xs[:], 0.0)       # ReLU
                    stats = xprep.tile([128, 6], F32, tag="stats")
                    nc.vector.bn_stats(stats[:], xs[:])
                    mv = xprep.tile([128, 2], F32, tag="mv")
                    nc.vector.bn_aggr(mv[:], stats[:])
                    lnv = xprep.tile([128, 1], F32, tag="lnv")
                    nc.scalar.activation(lnv[:], mv[:, 1:2], Ln, bias=eps_t[:, 0:1])
                    rstd = xprep.tile([128, 1], F32, tag="rstd")
                    nc.scalar.activation(rstd[:], lnv[:], Exp, scale=-0.5)
                    xn = xprep.tile([128, C], F32, tag="xn")
                    nc.vector.tensor_scalar(
                        xn[:], xs[:], mv[:, 0:1], rstd[:, 0:1], op0=sub, op1=mult
                    )
                    nc.vector.tensor_mul(xn[:], xn[:], bgb[:, C:2 * C])  # * gamma
                    nc.vector.tensor_add(xn[:], xn[:], bgb[:, 2 * C:3 * C])  # + beta
                    tp = psp.tile([C, 128], F32, tag="tp")
                    nc.tensor.transpose(tp[:], xn[:], ident[:])
                    nc.vector.tensor_copy(xT[:, t * 128:(t + 1) * 128], tp[:])
                # second warmup burst: executes right after the prep
                # matmuls, covering the xTcat bounce window so the PE has
                # no >3us idle before the product matmuls begin
                for _ in range(12):
                    nc.tensor.matmul(warm[:], dummy[0:1, 0:128], dummy[0:1, :])

            # concatenate all xT rows onto partition 0 (matmul operands must
            # have base partition 0/32/64): SBUF -> DRAM -> SBUF bounce
            xTcat = constp.tile([1, C * N], BF16)
            nc.sync.dma_start(xstage_d[:], xT[:])
            nc.sync.dma_start(
                xTcat[0:1, :], xstage_d[:].rearrange("a b -> (a b)")[None, :]
            )


            # ---- main: exp(x_i*x_j), softmax over c, store ----------------
            with (
                tc.tile_pool(name="main", bufs=1) as mainp,
                tc.tile_pool(name="scr", bufs=2) as scrp,
                tc.tile_pool(name="small", bufs=2) as smallp,
                tc.tile_pool(name="psum_bc", bufs=2, space=bass.MemorySpace.PSUM) as pbc,
            ):
                def partial_tree(expt, sc, g, jw):
                    """Sum channels [8g,8g+8) of expt into sc[:, g, 0:jw]."""
                    e = expt[:, 8 * g:8 * (g + 1), 0:jw]
                    s4 = scrp.tile([128, 4, N], BF16, tag="s4")
                    nc.vector.tensor_add(s4[:, :, 0:jw], e[:, 0:4, :], e[:, 4:8, :])
                    nc.vector.tensor_add(
                        s4[:, 0:2, 0:jw], s4[:, 0:2, 0:jw], s4[:, 2:4, 0:jw]
                    )
                    nc.vector.tensor_add(
                        sc[:, g, 0:jw], s4[:, 0, 0:jw], s4[:, 1, 0:jw]
                    )

                def finish(expt, sc, jw, rows):
                    """Combine partials, recip, normalize, store.
                    rows = list of (out_row0, j0) 128-j store groups."""
                    nc.vector.tensor_add(
                        sc[:, 0:4, 0:jw], sc[:, 0:4, 0:jw], sc[:, 4:8, 0:jw]
                    )
                    nc.vector.tensor_add(
                        sc[:, 0:2, 0:jw], sc[:, 0:2, 0:jw], sc[:, 2:4, 0:jw]
                    )
                    s32 = smallp.tile([128, N], F32, tag="s32")
                    nc.vector.tensor_add(
                        s32[:, 0:jw], sc[:, 0, 0:jw], sc[:, 1, 0:jw]
                    )
                    recip = smallp.tile([128, N], F32, tag="recip")
                    nc.vector.reciprocal_approx_fast(recip[:, 0:jw], s32[:, 0:jw])
                    rb = smallp.tile([128, N], BF16, tag="rb")
                    nc.vector.tensor_copy(rb[:, 0:jw], recip[:, 0:jw])
                    CC = 8
                    for q in range(C // CC):
                        cs = slice(q * CC, (q + 1) * CC)
                        nc.vector.tensor_mul(
                            expt[:, cs, 0:jw],
                            expt[:, cs, 0:jw],
                            rb[:, None, 0:jw].broadcast_to((128, CC, jw)),
                        )
                        for row0, j0 in rows:
                            nc.sync.dma_start(
                                out_d[row0:row0 + 128,
                                      q * CC * JW:(q + 1) * CC * JW]
                                .rearrange("p (c j) -> p c j", j=JW),
                                expt[:, cs, j0:j0 + JW],
                            )

                # --- i-tile 0: one [128, C, 512] tile covers units 0+1 ---
                e01 = mainp.tile([128, C, N], BF16, tag="exp01")
                sc01 = scrp.tile([128, 8, N], BF16, tag="sc")
                for r in range(16):
                    ps = pbc.tile([128, 4, N], F32, tag="ps")
                    for ci in range(4):
                        c = 4 * r + ci
                        nc.tensor.matmul(
                            ps[:, ci, :],
                            xTcat[0:1, c * N:c * N + 128],
                            xTcat[0:1, c * N:c * N + N],
                        )
                    nc.scalar.activation(e01[:, 4 * r:4 * (r + 1), :], ps[:], Exp)
                    if r % 2 == 1:
                        partial_tree(e01, sc01, r // 2, N)

                # all of finish01's vector work is ready at i-tile-0 end;
                # emit it BEFORE unit 2's partials so the vector queue never
                # head-of-line blocks on unit 2's activations
                fin01 = lambda: finish(e01, sc01, N, [(0, 0), (128, JW)])

                # --- i-tile 1: unit 2 (j 128:384), 8-channel rounds ---
                e2 = mainp.tile([128, C, JW], BF16, tag="exp2")
                sc2 = scrp.tile([128, 8, N], BF16, tag="sc")
                for r in range(8):
                    ps = pbc.tile([128, 8, JW], F32, tag="ps")
                    for ci in range(8):
                        c = 8 * r + ci
                        nc.tensor.matmul(
                            ps[:, ci, :],
                            xTcat[0:1, c * N + 128:c * N + 256],
                            xTcat[0:1, c * N + 128:c * N + 384],
                        )
                    nc.scalar.activation(e2[:, 8 * r:8 * (r + 1), :], ps[:], Exp)
                    if r == 0:
                        fin01()

                for g in range(8):
                    partial_tree(e2, sc2, g, JW)
                finish(e2, sc2, JW, [(256, 0)])
    nc.compile()
    return nc


def _in_maps(h, W, b, gamma, beta):
    h = np.asarray(h, dtype=np.float32)
    W = np.asarray(W, dtype=np.float32)
    b = np.asarray(b, dtype=np.float32)
    gamma = np.asarray(gamma, dtype=np.float32)
    beta = np.asarray(beta, dtype=np.float32)

    WT = np.ascontiguousarray(W.T)
    bgb = np.ascontiguousarray(
        np.broadcast_to(np.concatenate([b, gamma, beta])[None, :], (128, 3 * C))
    )
    ident = np.eye(128, dtype=np.float32)

    in_maps = []
    for k in range(NCORES):
        bb, half = divmod(k, 2)
        hloc = h[bb]
        if half == 1:  # odd core: tile order (3,2,1,0)
            hloc = hloc.reshape(4, 128, C)[::-1].reshape(N, C)
        in_maps.append({
            "hT": np.ascontiguousarray(hloc.T),
            "WT": WT,
            "bgb": bgb,
            "identity": ident,
        })
    return in_maps


def run(h, W, b, gamma, beta, trace=False, **trace_kwargs):
    if "nc" not in _CACHE:
        _CACHE["nc"] = _build_program()
    nc = _CACHE["nc"]
    res = run_bass_kernel_spmd(
        nc,
        _in_maps(h, W, b, gamma, beta),
        core_ids=list(range(NCORES)),
        trace=trace,
        **trace_kwargs,
    )
    out = np.zeros((B, N, N, C), dtype=np.float32)
    for bb in range(B):
        blocks = {}
        for half in (0, 1):
            buf = np.asarray(res.results[2 * bb + half]["out"]).astype(np.float32)
            # [3u, 128i, C, JW] -> [3u, 128i, JW j, C]
            arr = buf.reshape(NU, 128, C, JW).transpose(0, 1, 3, 2)
            # global (i-tile, j-tile) of each unit's two 128-j halves
            if half == 0:
                tiles = [(0, 0), (0, 1), (0, 2), (0, 3), (1, 1), (1, 2)]
            else:
                tiles = [(3, 3), (3, 2), (3, 1), (3, 0), (2, 2), (2, 1)]
            for u in range(NU):
                for s in range(2):
                    blocks[tiles[2 * u + s]] = arr[u][:, 128 * s:128 * (s + 1)]
        # symmetric completions
        blocks[(1, 0)] = blocks[(0, 1)].transpose(1, 0, 2)
        blocks[(2, 0)] = blocks[(0, 2)].transpose(1, 0, 2)
        blocks[(1, 3)] = blocks[(3, 1)].transpose(1, 0, 2)
        blocks[(2, 3)] = blocks[(3, 2)].transpose(1, 0, 2)
        for (ti, tj), blk in blocks.items():
            out[bb, 128 * ti:128 * (ti + 1), 128 * tj:128 * (tj + 1)] = blk
    return out, res


def kernel(h, W, b, gamma, beta):
    out, _ = run(h, W, b, gamma, beta)
    return out


# revision 25
# speedup vs baseline: 1.0478x; 1.0478x over previous
"""Trainium2 Bass kernel for nn_DistanceModule.

Computes, for h [4,512,64], W [64,64], b/gamma/beta [64]:
    x = LayerNorm(ReLU(h @ W.T + b))          # [B,N,C]
    D[b,i,j,c] = x[b,i,c] * x[b,j,c]
    out = softmax(D, axis=-1)                 # [B,N,N,C] f32 (256 MB)

Sharding + symmetry: out[b,i,j,c] == out[b,j,i,c] exactly (the product
commutes), so of each batch's 4x4 grid of 128x128 (i,j) blocks only 12
need computing. Core pair (2b, 2b+1): even core takes tile order
(0,1,2,3), odd core (3,2,1,0) -- the SAME program in local tile
coords computes units [(it0,j 0:256), (it0,j 256:512), (it1,j 128:384)]
on both, which lands on blocks rows 0-1 (even) / rows 3-2 (odd).
The host places 12 blocks directly and 4 as transposes.

Per-core pipeline, c-major on-chip layout (contiguous APs everywhere):
  PE     : K=1 outer-product bf16 matmuls (lhsT = xTcat[0, c*N+i0]
           [1,128], rhs = xTcat[0, c*N+j0] [1,512|256]) write logit
           x_i[c]*x_j[c] into PSUM. All x rows are concatenated on
           partition 0 (matmul base-partition rule) via a DRAM bounce.
  ScalarE: contiguous exp activations (FD=1024, PSUM src, bf16 dst).
           Only Ln/Exp are used on ScalarE (single table set).
  VectorE: channel sums via bf16 add-trees in 16-channel partial
           groups (tensor_tensor 2x 16-bit mode), emitted as the exp
           tiles fill so VectorE overlaps the fill; then
           reciprocal_approx_fast and an in-place bf16 normalize
           multiply against a stride-0-broadcast reciprocal.
  DMA    : contiguous bf16 stores in (c,j) order; the host transposes
           blocks to (j,c) while casting back to f32.

Softmax needs no max-subtraction: LayerNorm bounds |x| by sqrt(C-1),
logits <= 63, exp <= 2.4e27 which fits bf16 range.
Measured rel err ~5e-3 vs f32 reference (harness gate 2e-2).
"""

import numpy as np

import concourse.bacc as bacc
import concourse.bass as bass
import concourse.mybir as mybir
import concourse.tile as tile
from concourse.bass_utils import run_bass_kernel_spmd

B, N, C = 4, 512, 64
NCORES = 8
ROWS = 256
JW = 256            # unit j-width
NU = 3              # units per core
EPS = 1e-5
F32 = mybir.dt.float32
BF16 = mybir.dt.bfloat16

_CACHE = {}


def _build_program():
    nc = bacc.Bacc(
        "TRN2",
        target_bir_lowering=False,
        debug=False,
        enable_asserts=False,
        num_devices=NCORES,
    )

    hT_d = nc.dram_tensor("hT", [C, N], F32, kind="ExternalInput")
    WT_d = nc.dram_tensor("WT", [C, C], F32, kind="ExternalInput")
    bgb_d = nc.dram_tensor("bgb", [128, 3 * C], F32, kind="ExternalInput")
    id_d = nc.dram_tensor("identity", [128, 128], F32, kind="ExternalInput")
    xstage_d = nc.dram_tensor("xstage", [C, N], BF16, kind="Internal")
    # unit u -> rows [u*128,(u+1)*128), (c,j)-major columns
    out_d = nc.dram_tensor("out", [NU * 128, C * JW], BF16, kind="ExternalOutput")

    sub = mybir.AluOpType.subtract
    mult = mybir.AluOpType.mult
    Exp = mybir.ActivationFunctionType.Exp
    Ln = mybir.ActivationFunctionType.Ln

    with tile.TileContext(nc) as tc:
        with tc.tile_pool(name="const", bufs=1) as constp:
            hT = constp.tile([C, N], F32)
            nc.sync.dma_start(hT[:], hT_d[:])
            WT = constp.tile([C, C], F32)
            nc.sync.dma_start(WT[:], WT_d[:])
            bgb = constp.tile([128, 3 * C], F32)
            nc.sync.dma_start(bgb[:], bgb_d[:])
            ident = constp.tile([128, 128], F32)
            nc.sync.dma_start(ident[:], id_d[:])

            xT = constp.tile([C, N], BF16)
            eps_t = constp.tile([128, 1], F32)
            nc.vector.memset(eps_t[:], EPS)


            # ---- x = LayerNorm(ReLU(h @ W.T + b)), transposed to bf16 ----
            # ReLU on VectorE; rstd = exp(-0.5*ln(var+eps)) keeps ScalarE in
            # the natural_log_exp table set (one ACT_TABLE_LOAD).
            with (
                tc.tile_pool(name="xprep", bufs=2) as xprep,
                tc.tile_pool(name="psum_prep", bufs=2, space=bass.MemorySpace.PSUM) as psp,
            ):
                for t in range(4):
                    xp = psp.tile([128, C], F32, tag="xp")
                    nc.tensor.matmul(xp[:], hT[:, t * 128:(t + 1) * 128], WT[:])
                    xs = xprep.tile([128, C], F32, tag="xs")
                    nc.vector.tensor_add(xs[:], xp[:], bgb[:, 0:C])      # + b
                    nc.vector.tensor_scalar_max(xs[:], xs[:], 0.0)       # ReLU
                    stats = xprep.tile([128, 6], F32, tag="stats")
                    nc.vector.bn_stats(stats[:], xs[:])
                    mv = xprep.tile([128, 2], F32, tag="mv")
                    nc.vector.bn_aggr(mv[:], stats[:])
                    lnv = xprep.tile([128, 1], F32, tag="lnv")
                    nc.scalar.activation(lnv[:], mv[:, 1:2], Ln, bias=eps_t[:, 0:1])
                    rstd = xprep.tile([128, 1], F32, tag="rstd")
                    nc.scalar.activation(rstd[:], lnv[:], Exp, scale=-0.5)
                    xn = xprep.tile([128, C], F32, tag="xn")
                    nc.vector.tensor_scalar(
                        xn[:], xs[:], mv[:, 0:1], rstd[:, 0:1], op0=sub, op1=mult
                    )
                    nc.vector.tensor_mul(xn[:], xn[:], bgb[:, C:2 * C])  # * gamma
                    nc.vector.tensor_add(xn[:], xn[:], bgb[:, 2 * C:3 * C])  # + beta
                    tp = psp.tile([C, 128], F32, tag="tp")
                    nc.tensor.transpose(tp[:], xn[:], ident[:])
                    nc.vector.tensor_copy(xT[:, t * 128:(t + 1) * 128], tp[:])

            # concatenate all xT rows onto partition 0 (matmul operands must
            # have base partition 0/32/64): SBUF -> DRAM -> SBUF bounce
            xTcat = constp.tile([1, C * N], BF16)
            nc.sync.dma_start(xstage_d[:], xT[:])
            nc.sync.dma_start(
                xTcat[0:1, :], xstage_d[:].rearrange("a b -> (a b)")[None, :]
            )


            # ---- main: exp(x_i*x_j), softmax over c, store ----------------
            with (
                tc.tile_pool(name="main", bufs=1) as mainp,
                tc.tile_pool(name="scr", bufs=2) as scrp,
                tc.tile_pool(name="small", bufs=2) as smallp,
                tc.tile_pool(name="psum_bc", bufs=2, space=bass.MemorySpace.PSUM) as pbc,
            ):
                def partial_tree(expt, sc, g, jw):
                    """Sum channels [8g,8g+8) of expt into sc[:, g, 0:jw]."""
                    e = expt[:, 8 * g:8 * (g + 1), 0:jw]
                    s4 = scrp.tile([128, 4, N], BF16, tag="s4")
                    nc.vector.tensor_add(s4[:, :, 0:jw], e[:, 0:4, :], e[:, 4:8, :])
                    nc.vector.tensor_add(
                        s4[:, 0:2, 0:jw], s4[:, 0:2, 0:jw], s4[:, 2:4, 0:jw]
                    )
                    nc.vector.tensor_add(
                        sc[:, g, 0:jw], s4[:, 0, 0:jw], s4[:, 1, 0:jw]
                    )

                def finish(expt, sc, jw, rows):
                    """Combine partials, recip, normalize, store.
                    rows = list of (out_row0, j0) 128-j store groups."""
                    nc.vector.tensor_add(
                        sc[:, 0:4, 0:jw], sc[:, 0:4, 0:jw], sc[:, 4:8, 0:jw]
                    )
                    nc.vector.tensor_add(
                        sc[:, 0:2, 0:jw], sc[:, 0:2, 0:jw], sc[:, 2:4, 0:jw]
                    )
                    s32 = smallp.tile([128, N], F32, tag="s32")
                    nc.vector.tensor_add(
                        s32[:, 0:jw], sc[:, 0, 0:jw], sc[:, 1, 0:jw]
                    )
                    recip = smallp.tile([128, N], F32, tag="recip")
                    nc.vector.reciprocal_approx_fast(recip[:, 0:jw], s32[:, 0:jw])
                    rb = smallp.tile([128, N], BF16, tag="rb")
                    nc.vector.tensor_copy(rb[:, 0:jw], recip[:, 0:jw])
                    CC = 8
                    for q in range(C // CC):
                        cs = slice(q * CC, (q + 1) * CC)
                        nc.vector.tensor_mul(
                            expt[:, cs, 0:jw],
                            expt[:, cs, 0:jw],
                            rb[:, None, 0:jw].broadcast_to((128, CC, jw)),
                        )
                        for row0, j0 in rows:
                            nc.sync.dma_start(
                                out_d[row0:row0 + 128,
                                      q * CC * JW:(q + 1) * CC * JW]
                                .rearrange("p (c j) -> p c j", j=JW),
                                expt[:, cs, j0:j0 + JW],
                            )

                # --- i-tile 0: one [128, C, 512] tile covers units 0+1 ---
                e01 = mainp.tile([128, C, N], BF16, tag="exp01")
                sc01 = scrp.tile([128, 8, N], BF16, tag="sc")
                for r in range(16):
                    ps = pbc.tile([128, 4, N], F32, tag="ps")
                    for ci in range(4):
                        c = 4 * r + ci
                        nc.tensor.matmul(
                            ps[:, ci, :],
                            xTcat[0:1, c * N:c * N + 128],
                            xTcat[0:1, c * N:c * N + N],
                        )
                    nc.scalar.activation(e01[:, 4 * r:4 * (r + 1), :], ps[:], Exp)
                    if r % 2 == 1:
                        partial_tree(e01, sc01, r // 2, N)

                # all of finish01's vector work is ready at i-tile-0 end;
                # emit it BEFORE unit 2's partials so the vector queue never
                # head-of-line blocks on unit 2's activations
                fin01 = lambda: finish(e01, sc01, N, [(0, 0), (128, JW)])

                # --- i-tile 1: unit 2 (j 128:384), 8-channel rounds ---
                e2 = mainp.tile([128, C, JW], BF16, tag="exp2")
                sc2 = scrp.tile([128, 8, N], BF16, tag="sc")
                for r in range(8):
                    ps = pbc.tile([128, 8, JW], F32, tag="ps")
                    for ci in range(8):
                        c = 8 * r + ci
                        nc.tensor.matmul(
                            ps[:, ci, :],
                            xTcat[0:1, c * N + 128:c * N + 256],
                            xTcat[0:1, c * N + 128:c * N + 384],
                        )
                    nc.scalar.activation(e2[:, 8 * r:8 * (r + 1), :], ps[:], Exp)
                    if r == 0:
                        fin01()

                for g in range(8):
                    partial_tree(e2, sc2, g, JW)
                finish(e2, sc2, JW, [(256, 0)])
    nc.compile()
    return nc


def _in_maps(h, W, b, gamma, beta):
    h = np.asarray(h, dtype=np.float32)
    W = np.asarray(W, dtype=np.float32)
    b = np.asarray(b, dtype=np.float32)
    gamma = np.asarray(gamma, dtype=np.float32)
    beta = np.asarray(beta, dtype=np.float32)

    WT = np.ascontiguousarray(W.T)
    bgb = np.ascontiguousarray(
        np.broadcast_to(np.concatenate([b, gamma, beta])[None, :], (128, 3 * C))
    )
    ident = np.eye(128, dtype=np.float32)

    in_maps = []
    for k in range(NCORES):
        bb, half = divmod(k, 2)
        hloc = h[bb]
        if half == 1:  # odd core: tile order (3,2,1,0)
            hloc = hloc.reshape(4, 128, C)[::-1].reshape(N, C)
        in_maps.append({
            "hT": np.ascontiguousarray(hloc.T),
            "WT": WT,
            "bgb": bgb,
            "identity": ident,
        })
    return in_maps


def run(h, W, b, gamma, beta, trace=False, **trace_kwargs):
    if "nc" not in _CACHE:
        _CACHE["nc"] = _build_program()
    nc = _CACHE["nc"]
    res = run_bass_kernel_spmd(
        nc,
        _in_maps(h, W, b, gamma, beta),
        core_ids=list(range(NCORES)),
        trace=trace,
        **trace_kwargs,
    )
    out = np.zeros((B, N, N, C), dtype=np.float32)
    for bb in range(B):
        blocks = {}
        for half in (0, 1):
            buf = np.asarray(res.results[2 * bb + half]["out"]).astype(np.float32)
            # [3u, 128i, C, JW] -> [3u, 128i, JW j, C]
            arr = buf.reshape(NU, 128, C, JW).transpose(0, 1, 3, 2)
            # global (i-tile, j-tile) of each unit's two 128-j halves
            if half == 0:
                tiles = [(0, 0), (0, 1), (0, 2), (0, 3), (1, 1), (1, 2)]
            else:
                tiles = [(3, 3), (3, 2), (3, 1), (3, 0), (2, 2), (2, 1)]
            for u in range(NU):
                for s in range(2):
                    blocks[tiles[2 * u + s]] = arr[u][:, 128 * s:128 * (s + 1)]
        # symmetric completions
        blocks[(1, 0)] = blocks[(0, 1)].transpose(1, 0, 2)
        blocks[(2, 0)] = blocks[(0, 2)].transpose(1, 0, 2)
        blocks[(1, 3)] = blocks[(3, 1)].transpose(1, 0, 2)
        blocks[(2, 3)] = blocks[(3, 2)].transpose(1, 0, 2)
        for (ti, tj), blk in blocks.items():
            out[bb, 128 * ti:128 * (ti + 1), 128 * tj:128 * (tj + 1)] = blk
    return out, res


def kernel(h, W, b, gamma, beta):
    out, _ = run(h, W, b, gamma, beta)
    return out
